# revision 24
# baseline (speedup 1.0000x reference)
"""ConvLSTM neck kernel for Trainium2 (8 NeuronCores, data-parallel over batch).

Problem: 2-layer ConvLSTM, B=8, T=12, C=HID=96, H=W=48, 3x3 SAME gate conv.
Sharding: batch across the 8 cores (B=1 per core); recurrence over T local.

Per core, per (t, layer): gates[384, 48x48] accumulate in PSUM from float32r
matmuls (1 cycle/row on the PE for free-dim >= 256, ~1e-4 rel err) over the
conv taps. The 192-channel contraction is split 128 + 64 to fill the PE:

  - xz tile [128, 50*50]: partitions 0:96 = layer input x, 96:128 = h ch 0:32
    -> one K=128 matmul per tap against W[cin 0:128] (9 taps)
  - tail tile [128, 50*50]: h ch 32:96 at partitions 0:64, and the SAME image
    shifted left by one column at partitions 64:128. A K=128 matmul at row
    offset dy then covers taps (dy,0) AND (dy,1) in one pass; tap (dy,2) is a
    K=64 matmul on partitions 0:64 with a +2 column AP offset. 15 matmuls of
    streaming per gate chunk instead of 18.

Zero-padded borders implement SAME padding; conv taps are just AP offsets.
Gate nonlinearities run on the scalar engine reading PSUM directly
(cross-partition-base activations), LSTM cell update on the vector engine,
which also scatters new h into the xz/tail layouts for the next step.
Layer 1's input is layer 0's h of the same timestep.
"""

import sys

for _p in ("/opt/trn_rl_repo",):
    if _p not in sys.path:
        sys.path.insert(0, _p)

from contextlib import ExitStack

import numpy as np

import concourse.bass as bass
import concourse.tile as tile
from concourse import mybir
from concourse.bass_utils import run_bass_kernel_spmd

F32 = mybir.dt.float32
F32R = mybir.dt.float32r
ACTF = mybir.ActivationFunctionType

L = 2
HID = 96
C = 96
T = 12
H = 48
Wd = 48
HW = H * Wd            # 2304
PW = Wd + 2            # 50
PHW = (H + 2) * PW     # 2500
NCORES = 8
SLABS = [(0, 10), (10, 10), (20, 10), (30, 10), (40, 8)]  # (row0, nrows)
WAFREE = L * 9 * 384   # head weight free size: (layer, tap, cout)
WBFREE = 5 * L * 384  # tail groups: 3 dy-pairs, (1,2)+(2,2) pair, (0,2) single


def split_multi_waits(nc):
    """This walrus accepts at most ONE sync-wait per instruction. Hoist extra
    waits onto standalone EventSemaphore instructions inserted just before, on
    the same engine queue (FIFO => identical semantics)."""
    n_split = 0
    for f in nc.m.functions:
        for bb in f.blocks:
            insts = bb.instructions
            i = 0
            while i < len(insts):
                inst = insts[i]
                si = inst.sync_info
                waits = list(si.on_wait) if si and si.on_wait else []
                if len(waits) > 1:
                    for k, w in enumerate(waits[:-1]):
                        ev = mybir.InstEventSemaphore(
                            name=f"{inst.name}-ws{k}",
                            sync_info=mybir.SyncInfo(on_wait=[w], on_update=[]),
                        )
                        ev.engine = inst.engine
                        insts.insert(i, ev)
                        i += 1
                        n_split += 1
                    si.on_wait = [waits[-1]]
                i += 1
    return n_split


def _build():
    nc = bass.Bass("TRN2", target_bir_lowering=False, debug=False)

    inp_d = nc.dram_tensor("inp", [T, C, HW], F32R, kind="ExternalInput").ap()
    xzi_d = nc.dram_tensor("xzi", [L, 128, PHW], F32R, kind="ExternalInput").ap()
    t1i_d = nc.dram_tensor("t1i", [L, 128, PHW], F32R, kind="ExternalInput").ap()
    t2i_d = nc.dram_tensor("t2i", [L, 128, PHW], F32R, kind="ExternalInput").ap()
    c0_d = nc.dram_tensor("c0", [L, HID, HW], F32, kind="ExternalInput").ap()
    wa_d = nc.dram_tensor("Wa", [128, WAFREE], F32R, kind="ExternalInput").ap()
    wb_d = nc.dram_tensor("Wb", [128, WBFREE], F32R, kind="ExternalInput").ap()
    b_d = nc.dram_tensor("br", [128, L * 3], F32, kind="ExternalInput").ap()
    z_d = nc.dram_tensor("zeros", [128, PHW], F32R, kind="ExternalInput").ap()

    outh_d = nc.dram_tensor("out_h", [L, T, HID, HW], F32R, kind="ExternalOutput").ap()
    outc_d = nc.dram_tensor("out_c", [L, HID, HW], F32, kind="ExternalOutput").ap()

    with tile.TileContext(nc) as tc, ExitStack() as ctx:
        const = ctx.enter_context(tc.tile_pool(name="const", bufs=1))
        temps = ctx.enter_context(tc.tile_pool(name="temps", bufs=2))
        psum = ctx.enter_context(tc.tile_pool(name="psum", bufs=2, space="PSUM"))

        wa_sb = const.tile([128, WAFREE], F32R, tag="wa_sb")
        wb_sb = const.tile([128, WBFREE], F32R, tag="wb_sb")
        b_sb = const.tile([128, L * 3], F32, tag="b_sb")
        # xz[l][i]: partitions 0:96 = layer input, 96:128 = own h ch 0:32
        xz = [
            [
                const.tile([128, PHW], F32R, name=f"xz{l}_{i}", tag=f"xz{l}_{i}")
                for i in range(2)
            ]
            for l in range(L)
        ]
        # tail[l][i]: h ch 32:96 at partitions 0:64 and again at 64:128
        tail = [
            [
                const.tile([128, PHW], F32R, name=f"tail{l}_{i}", tag=f"tail{l}_{i}")
                for i in range(2)
            ]
            for l in range(L)
        ]
        c_sb = [
            const.tile([HID, HW], F32, name=f"c_sb{l}", tag=f"c_sb{l}")
            for l in range(L)
        ]
        # tail2[l] (T2, single-buffered): h ch 32:96 shifted up 1 row at
        # partitions 0:64 and up 2 rows at 64:128 -> one K=128 matmul covers
        # taps (1,2)+(2,2)
        tail2 = [
            const.tile([128, PHW], F32R, name=f"tail2{l}", tag=f"tail2{l}")
            for l in range(L)
        ]

        nc.sync.dma_start(out=wa_sb[:], in_=wa_d)
        nc.sync.dma_start(out=wb_sb[:], in_=wb_d)
        nc.sync.dma_start(out=b_sb[:], in_=b_d)

        # t=0 state (x(0), padded/shifted h0 images, zero borders) is built
        # on the host and loaded with one contiguous DMA per tile; only the
        # buffers first read at t=1 get plain zero fills (borders persist --
        # interiors are rewritten every step)
        for l in range(L):
            nc.sync.dma_start(out=xz[l][0][:], in_=xzi_d[l])
            nc.sync.dma_start(out=tail[l][1][:], in_=t1i_d[l])
            nc.sync.dma_start(out=tail2[l][:], in_=t2i_d[l])
            nc.gpsimd.dma_start(out=xz[l][1][:], in_=z_d)
            nc.gpsimd.dma_start(out=tail[l][0][:], in_=z_d)

        def interior(tile_ap, p0, p1, nrows=H, row0=0, c0=1):
            # AP over the interior of a padded [*, 2500] tile:
            # [parts p0:p1][nrows rows, stride 50][48 cols starting at c0]
            # (c0=0 stores a copy shifted LEFT by one column)
            v = tile_ap.rearrange("p (r c) -> p r c", c=PW)
            return v[p0:p1, 1 + row0 : 1 + row0 + nrows, c0 : c0 + Wd]

        for l in range(L):
            nc.gpsimd.dma_start(out=c_sb[l][:], in_=c0_d[l])

        for t in range(T):
            cur, prev = t % 2, (t + 1) % 2
            if t > 0:
                nc.gpsimd.dma_start(out=interior(xz[0][cur][:], 0, 96), in_=inp_d[t])

            for l in range(L):
                xz_rd = xz[l][cur]
                tail_rd = tail[l][prev]
                bcol = [l * 3 + m for m in range(3)]

                for row0, nrows in SLABS:
                    n = nrows * Wd
                    g = [
                        psum.tile(
                            [128, n], F32, name=f"g{m}_{t}_{l}_{row0}", tag=f"g{m}"
                        )
                        for m in range(3)
                    ]
                    xv = xz_rd[:].rearrange("p (r c) -> p r c", c=PW)
                    tv = tail_rd[:].rearrange("p (r c) -> p r c", c=PW)
                    tv2 = tail2[l][:].rearrange("p (r c) -> p r c", c=PW)

                    for m in range(3):
                        # 9 K=128 matmuls (x + h ch 0:32), then 9 K=64 tail
                        # matmuls alternating PE row-groups 0:64 / 64:128 so
                        # consecutive pairs run concurrently in the array.
                        for off in range(9):
                            dy, dx = off // 3, off % 3
                            rhs = xv[:, row0 + dy : row0 + dy + nrows, dx : dx + Wd]
                            wo = (l * 9 + off) * 384 + m * 128
                            nc.tensor.matmul(
                                g[m][:],
                                wa_sb[:, wo : wo + 128],
                                rhs,
                                start=(off == 0),
                                stop=False,
                            )
                        for dy in range(3):
                            # taps (dy,0)+(dy,1) in one K=128 matmul: tail
                            # partitions 64:128 hold the column-shifted image
                            rhs = tv[:, row0 + dy : row0 + dy + nrows, 0:Wd]
                            wo = (l * 5 + dy) * 384 + m * 128
                            nc.tensor.matmul(
                                g[m][:],
                                wb_sb[:, wo : wo + 128],
                                rhs,
                                start=False,
                                stop=False,
                            )
                        # taps (1,2)+(2,2) in one K=128 matmul on T2
                        rhs = tv2[:, row0 : row0 + nrows, 2 : 2 + Wd]
                        wo = (l * 5 + 3) * 384 + m * 128
                        nc.tensor.matmul(
                            g[m][:], wb_sb[:, wo : wo + 128], rhs,
                            start=False, stop=False,
                        )
                        # tap (0,2): K=64 on T1's unshifted half
                        rhs = tv[0:64, row0 : row0 + nrows, 2 : 2 + Wd]
                        wo = (l * 5 + 4) * 384 + m * 128
                        nc.tensor.matmul(
                            g[m][:], wb_sb[0:64, wo : wo + 128], rhs,
                            start=False, stop=True,
                        )

                    # gate nonlinearities: out = func(in + bias).
                    # gate channel ranges inside the 3x128 psum chunks:
                    #   i = g0[0:96], f = g0[96:128] + g1[0:64],
                    #   o = g1[64:128] + g2[0:32], g = g2[32:128]
                    i_s = temps.tile([HID, n], F32, tag="i_s")
                    f_s = temps.tile([HID, n], F32, tag="f_s")
                    o_s = temps.tile([HID, n], F32, tag="o_s")
                    g_t = temps.tile([HID, n], F32, tag="g_t")
                    c_t = temps.tile([HID, n], F32, tag="c_t")
                    ig = temps.tile([HID, n], F32, tag="ig")

                    def act(dst, src_g, gm, p_in0, p_out0, cnt, func):
                        nc.scalar.activation(
                            out=dst[p_out0 : p_out0 + cnt, :],
                            in_=src_g[p_in0 : p_in0 + cnt, :],
                            func=func,
                            bias=b_sb[p_in0 : p_in0 + cnt, bcol[gm] : bcol[gm] + 1],
                        )

                    act(i_s, g[0], 0, 0, 0, 96, ACTF.Sigmoid)
                    act(f_s, g[0], 0, 96, 0, 32, ACTF.Sigmoid)
                    act(f_s, g[1], 1, 0, 32, 32, ACTF.Sigmoid)
                    act(f_s, g[1], 1, 32, 64, 32, ACTF.Sigmoid)
                    act(o_s, g[1], 1, 64, 0, 64, ACTF.Sigmoid)
                    act(o_s, g[2], 2, 0, 64, 32, ACTF.Sigmoid)
                    act(g_t, g[2], 2, 32, 0, 32, ACTF.Tanh)
                    act(g_t, g[2], 2, 64, 32, 32, ACTF.Tanh)
                    act(g_t, g[2], 2, 96, 64, 32, ACTF.Tanh)

                    c_sl = c_sb[l][:, row0 * Wd : row0 * Wd + n]
                    nc.vector.tensor_mul(ig[:], i_s[:], g_t[:])
                    nc.vector.tensor_mul(i_s[:], f_s[:], c_sl)  # i_s := f*c_old
                    nc.vector.tensor_add(c_sl, i_s[:], ig[:])   # c_new
                    nc.scalar.activation(out=c_t[:], in_=c_sl, func=ACTF.Tanh)

                    # h = o * tanh(c), scattered into next-step matmul layouts:
                    #   ch 0:32  -> xz[l][next] partitions 96:128
                    #   ch 32:96 -> tail[l][cur] partitions 0:64 and 64:128
                    #   (l=0 only) full h -> xz[1][cur] partitions 0:96
                    def hmul(dst, p_dst0, p_src0, cnt, c0=1, r_shift=0):
                        r0 = row0 + r_shift
                        drop = -r0 - 1 if r0 < -1 else 0
                        nc.vector.tensor_mul(
                            interior(
                                dst[:], p_dst0, p_dst0 + cnt,
                                nrows=nrows - drop, row0=r0 + drop, c0=c0,
                            ),
                            o_s[p_src0 : p_src0 + cnt, drop * Wd : n],
                            c_t[p_src0 : p_src0 + cnt, drop * Wd : n],
                        )

                    if l == 0:
                        hmul(xz[1][cur], 0, 0, 96)
                    if t < T - 1:
                        hmul(xz[l][prev], 96, 0, 32)
                        hmul(tail[l][cur], 0, 32, 32)
                        hmul(tail[l][cur], 32, 64, 32)
                        hmul(tail[l][cur], 64, 32, 32, c0=0)
                        hmul(tail[l][cur], 96, 64, 32, c0=0)
                        hmul(tail2[l], 0, 32, 32, r_shift=-1)
                        hmul(tail2[l], 32, 64, 32, r_shift=-1)
                        hmul(tail2[l], 64, 32, 32, r_shift=-2)
                        hmul(tail2[l], 96, 64, 32, r_shift=-2)
                    else:
                        # last step: no next-step recurrence consumers; emit h
                        # once to a contiguous staging tile so the final
                        # output store is descriptor-cheap and the tail short
                        h_st = temps.tile([HID, n], F32R, tag="h_st")
                        nc.vector.tensor_mul(h_st[:], o_s[:], c_t[:])
                        nc.sync.dma_start(
                            out=outh_d[l, t, :, row0 * Wd : row0 * Wd + n],
                            in_=h_st[:],
                        )

                # all_hidden[l, t]: ch 0:32 from xz[l][next] p96:128,
                # ch 32:96 from tail[l][cur] p0:64 (last step staged per slab)
                if t < T - 1:
                    nc.sync.dma_start(
                        out=outh_d[l, t, 0:32], in_=interior(xz[l][prev][:], 96, 128)
                    )
                    nc.sync.dma_start(
                        out=outh_d[l, t, 32:96], in_=interior(tail[l][cur][:], 0, 64)
                    )

        for l in range(L):
            nc.sync.dma_start(out=outc_d[l], in_=c_sb[l][:])

    split_multi_waits(nc)
    return nc


_NC = None


def _get_nc():
    global _NC
    if _NC is None:
        _NC = _build()
    return _NC


def make_inmaps(inp, h0, c0, W, b):
    inp = np.ascontiguousarray(inp, dtype=np.float32)  # [8, 12, 96, 48, 48]
    h0 = np.ascontiguousarray(h0, dtype=np.float32)    # [2, 8, 96, 48, 48]
    c0 = np.ascontiguousarray(c0, dtype=np.float32)
    W = np.ascontiguousarray(W, dtype=np.float32)      # [2, 384, 192, 3, 3]
    b = np.ascontiguousarray(b, dtype=np.float32)      # [2, 384]

    # weights -> lhsT layout; wt6: [cin, l, dy, dx, cout]
    wt6 = W.transpose(2, 0, 3, 4, 1)
    w_a = np.ascontiguousarray(wt6[0:128].reshape(128, WAFREE))
    # tail weight groups, free layout (l*5+g)*384+co:
    #   g=0..2: dy-pair -> lower (dy,0), upper (dy,1)
    #   g=3:    T2 pair -> lower (1,2),  upper (2,2)
    #   g=4:    single  -> lower (0,2)
    w_b = np.zeros((128, WBFREE), dtype=np.float32)
    wb5 = w_b.reshape(128, L, 5, 384)
    wb5[0:64, :, 0:3] = wt6[128:192, :, :, 0, :]
    wb5[64:128, :, 0:3] = wt6[128:192, :, :, 1, :]
    wb5[0:64, :, 3] = wt6[128:192, :, 1, 2, :]
    wb5[64:128, :, 3] = wt6[128:192, :, 2, 2, :]
    wb5[0:64, :, 4] = wt6[128:192, :, 0, 2, :]
    # bias -> [partition, (l, cout_chunk)]
    b_r = np.ascontiguousarray(
        b.reshape(L, 3, 128).transpose(2, 0, 1).reshape(128, L * 3)
    )
    zeros = np.zeros((128, PHW), dtype=np.float32)

    def padded(a):  # [ch, 48, 48] -> [ch, 50, 50] zero-padded
        return np.pad(a, ((0, 0), (1, 1), (1, 1)))

    def initial_tiles(bb):
        h0b = h0[:, bb]  # [L, 96, 48, 48]
        xzi = np.zeros((L, 128, H + 2, PW), dtype=np.float32)
        t1i = np.zeros((L, 128, H + 2, PW), dtype=np.float32)
        t2i = np.zeros((L, 128, H + 2, PW), dtype=np.float32)
        xzi[0, 0:96] = padded(inp[bb, 0])
        for l in range(L):
            xzi[l, 96:128] = padded(h0b[l, 0:32])
            tl = padded(h0b[l, 32:96])
            t1i[l, 0:64] = tl
            t1i[l, 64:128, :, 0 : PW - 1] = tl[:, :, 1:PW]      # col-shift 1
            t2i[l, 0:64, 0 : H + 1, :] = tl[:, 1 : H + 2, :]    # row-shift 1
            t2i[l, 64:128, 0:H, :] = tl[:, 2 : H + 2, :]        # row-shift 2
        return (
            xzi.reshape(L, 128, PHW),
            t1i.reshape(L, 128, PHW),
            t2i.reshape(L, 128, PHW),
        )

    inits = [initial_tiles(bb) for bb in range(NCORES)]

    return [
        {
            "inp": inp[bb].reshape(T, C, HW),
            "xzi": inits[bb][0],
            "t1i": inits[bb][1],
            "t2i": inits[bb][2],
            "c0": c0[:, bb].reshape(L, HID, HW),
            "Wa": w_a,
            "Wb": w_b,
            "br": b_r,
            "zeros": zeros,
        }
        for bb in range(NCORES)
    ]


def kernel(inp, h0, c0, W, b):
    in_maps = make_inmaps(inp, h0, c0, W, b)
    res = run_bass_kernel_spmd(_get_nc(), in_maps, list(range(NCORES)))

    all_hidden = np.stack(
        [res.results[bb]["out_h"] for bb in range(NCORES)], axis=1
    ).reshape(L, NCORES, T, HID, H, Wd)
    c_f = np.stack(
        [res.results[bb]["out_c"] for bb in range(NCORES)], axis=1
    ).reshape(L, NCORES, HID, H, Wd)
    h_f = np.ascontiguousarray(all_hidden[:, :, -1])
    return all_hidden, h_f, c_f


# revision 26
# speedup vs baseline: 1.0011x; 1.0011x over previous
"""ConvLSTM neck kernel for Trainium2 (8 NeuronCores, data-parallel over batch).

Problem: 2-layer ConvLSTM, B=8, T=12, C=HID=96, H=W=48, 3x3 SAME gate conv.
Sharding: batch across the 8 cores (B=1 per core); recurrence over T local.

Per core, per (t, layer): gates[384, 48x48] accumulate in PSUM from float32r
matmuls (1 cycle/row on the PE for free-dim >= 256, ~1e-4 rel err) over the
conv taps. The 192-channel contraction is split 128 + 64 to fill the PE:

  - xz tile [128, 50*50]: partitions 0:96 = layer input x, 96:128 = h ch 0:32
    -> one K=128 matmul per tap against W[cin 0:128] (9 taps)
  - tail tile [128, 50*50]: h ch 32:96 at partitions 0:64, and the SAME image
    shifted left by one column at partitions 64:128. A K=128 matmul at row
    offset dy then covers taps (dy,0) AND (dy,1) in one pass; tap (dy,2) is a
    K=64 matmul on partitions 0:64 with a +2 column AP offset. 15 matmuls of
    streaming per gate chunk instead of 18.

Zero-padded borders implement SAME padding; conv taps are just AP offsets.
Gate nonlinearities run on the scalar engine reading PSUM directly
(cross-partition-base activations), LSTM cell update on the vector engine,
which also scatters new h into the xz/tail layouts for the next step.
Layer 1's input is layer 0's h of the same timestep.
"""

import sys

for _p in ("/opt/trn_rl_repo",):
    if _p not in sys.path:
        sys.path.insert(0, _p)

from contextlib import ExitStack

import numpy as np

import concourse.bass as bass
import concourse.tile as tile
from concourse import mybir
from concourse.bass_utils import run_bass_kernel_spmd

F32 = mybir.dt.float32
F32R = mybir.dt.float32r
ACTF = mybir.ActivationFunctionType

L = 2
HID = 96
C = 96
T = 12
H = 48
Wd = 48
HW = H * Wd            # 2304
PW = Wd + 2            # 50
PHW = (H + 2) * PW     # 2500
NCORES = 8
SLABS = [(0, 10), (10, 10), (20, 10), (30, 10), (40, 8)]  # (row0, nrows)
WAFREE = L * 9 * 384   # head weight free size: (layer, tap, cout)
WBFREE = 5 * L * 384  # tail groups: 3 dy-pairs, (1,2)+(2,2) pair, (0,2) single


def split_multi_waits(nc):
    """This walrus accepts at most ONE sync-wait per instruction. Hoist extra
    waits onto standalone EventSemaphore instructions inserted just before, on
    the same engine queue (FIFO => identical semantics)."""
    n_split = 0
    for f in nc.m.functions:
        for bb in f.blocks:
            insts = bb.instructions
            i = 0
            while i < len(insts):
                inst = insts[i]
                si = inst.sync_info
                waits = list(si.on_wait) if si and si.on_wait else []
                if len(waits) > 1:
                    for k, w in enumerate(waits[:-1]):
                        ev = mybir.InstEventSemaphore(
                            name=f"{inst.name}-ws{k}",
                            sync_info=mybir.SyncInfo(on_wait=[w], on_update=[]),
                        )
                        ev.engine = inst.engine
                        insts.insert(i, ev)
                        i += 1
                        n_split += 1
                    si.on_wait = [waits[-1]]
                i += 1
    return n_split


def _build():
    nc = bass.Bass("TRN2", target_bir_lowering=False, debug=False)

    inp_d = nc.dram_tensor("inp", [T, C, HW], F32R, kind="ExternalInput").ap()
    xzi_d = nc.dram_tensor("xzi", [L, 128, PHW], F32R, kind="ExternalInput").ap()
    t1i_d = nc.dram_tensor("t1i", [L, 128, PHW], F32R, kind="ExternalInput").ap()
    t2i_d = nc.dram_tensor("t2i", [L, 128, PHW], F32R, kind="ExternalInput").ap()
    c0_d = nc.dram_tensor("c0", [L, HID, HW], F32, kind="ExternalInput").ap()
    wa_d = nc.dram_tensor("Wa", [128, WAFREE], F32R, kind="ExternalInput").ap()
    wb_d = nc.dram_tensor("Wb", [128, WBFREE], F32R, kind="ExternalInput").ap()
    b_d = nc.dram_tensor("br", [128, L * 3], F32, kind="ExternalInput").ap()
    z_d = nc.dram_tensor("zeros", [128, PHW], F32R, kind="ExternalInput").ap()

    outh_d = nc.dram_tensor("out_h", [L, T, HID, HW], F32R, kind="ExternalOutput").ap()
    outc_d = nc.dram_tensor("out_c", [L, HID, HW], F32, kind="ExternalOutput").ap()

    with tile.TileContext(nc) as tc, ExitStack() as ctx:
        const = ctx.enter_context(tc.tile_pool(name="const", bufs=1))
        temps = ctx.enter_context(tc.tile_pool(name="temps", bufs=2))
        psum = ctx.enter_context(tc.tile_pool(name="psum", bufs=2, space="PSUM"))

        wa_sb = const.tile([128, WAFREE], F32R, tag="wa_sb")
        wb_sb = const.tile([128, WBFREE], F32R, tag="wb_sb")
        b_sb = const.tile([128, L * 3], F32, tag="b_sb")
        # xz[l][i]: partitions 0:96 = layer input, 96:128 = own h ch 0:32
        xz = [
            [
                const.tile([128, PHW], F32R, name=f"xz{l}_{i}", tag=f"xz{l}_{i}")
                for i in range(2)
            ]
            for l in range(L)
        ]
        # tail[l][i]: h ch 32:96 at partitions 0:64 and again at 64:128
        tail = [
            [
                const.tile([128, PHW], F32R, name=f"tail{l}_{i}", tag=f"tail{l}_{i}")
                for i in range(2)
            ]
            for l in range(L)
        ]
        c_sb = [
            const.tile([HID, HW], F32, name=f"c_sb{l}", tag=f"c_sb{l}")
            for l in range(L)
        ]
        # tail2[l] (T2, single-buffered): h ch 32:96 shifted up 1 row at
        # partitions 0:64 and up 2 rows at 64:128 -> one K=128 matmul covers
        # taps (1,2)+(2,2)
        tail2 = [
            const.tile([128, PHW], F32R, name=f"tail2{l}", tag=f"tail2{l}")
            for l in range(L)
        ]

        nc.sync.dma_start(out=wa_sb[:], in_=wa_d)
        nc.sync.dma_start(out=wb_sb[:], in_=wb_d)
        nc.sync.dma_start(out=b_sb[:], in_=b_d)

        # t=0 state (x(0), padded/shifted h0 images, zero borders) is built
        # on the host and loaded with one contiguous DMA per tile; only the
        # buffers first read at t=1 get plain zero fills (borders persist --
        # interiors are rewritten every step)
        for l in range(L):
            nc.sync.dma_start(out=xz[l][0][:], in_=xzi_d[l])
            nc.sync.dma_start(out=tail[l][1][:], in_=t1i_d[l])
            nc.sync.dma_start(out=tail2[l][:], in_=t2i_d[l])
            nc.gpsimd.dma_start(out=xz[l][1][:], in_=z_d)
            nc.gpsimd.dma_start(out=tail[l][0][:], in_=z_d)

        def interior(tile_ap, p0, p1, nrows=H, row0=0, c0=1):
            # AP over the interior of a padded [*, 2500] tile:
            # [parts p0:p1][nrows rows, stride 50][48 cols starting at c0]
            # (c0=0 stores a copy shifted LEFT by one column)
            v = tile_ap.rearrange("p (r c) -> p r c", c=PW)
            return v[p0:p1, 1 + row0 : 1 + row0 + nrows, c0 : c0 + Wd]

        for l in range(L):
            nc.gpsimd.dma_start(out=c_sb[l][:], in_=c0_d[l])

        for t in range(T):
            cur, prev = t % 2, (t + 1) % 2
            if t > 0:
                nc.gpsimd.dma_start(out=interior(xz[0][cur][:], 0, 96), in_=inp_d[t])

            for l in range(L):
                xz_rd = xz[l][cur]
                tail_rd = tail[l][prev]
                bcol = [l * 3 + m for m in range(3)]

                for row0, nrows in SLABS:
                    n = nrows * Wd
                    g = [
                        psum.tile(
                            [128, n], F32, name=f"g{m}_{t}_{l}_{row0}", tag=f"g{m}"
                        )
                        for m in range(3)
                    ]
                    xv = xz_rd[:].rearrange("p (r c) -> p r c", c=PW)
                    tv = tail_rd[:].rearrange("p (r c) -> p r c", c=PW)
                    tv2 = tail2[l][:].rearrange("p (r c) -> p r c", c=PW)

                    for m in range(3):
                        # 9 K=128 matmuls (x + h ch 0:32), then 9 K=64 tail
                        # matmuls alternating PE row-groups 0:64 / 64:128 so
                        # consecutive pairs run concurrently in the array.
                        for off in range(9):
                            dy, dx = off // 3, off % 3
                            rhs = xv[:, row0 + dy : row0 + dy + nrows, dx : dx + Wd]
                            wo = (l * 9 + off) * 384 + m * 128
                            nc.tensor.matmul(
                                g[m][:],
                                wa_sb[:, wo : wo + 128],
                                rhs,
                                start=(off == 0),
                                stop=False,
                            )
                        for dy in range(3):
                            # taps (dy,0)+(dy,1) in one K=128 matmul: tail
                            # partitions 64:128 hold the column-shifted image
                            rhs = tv[:, row0 + dy : row0 + dy + nrows, 0:Wd]
                            wo = (l * 5 + dy) * 384 + m * 128
                            nc.tensor.matmul(
                                g[m][:],
                                wb_sb[:, wo : wo + 128],
                                rhs,
                                start=False,
                                stop=False,
                            )
                        # taps (1,2)+(2,2) in one K=128 matmul on T2
                        rhs = tv2[:, row0 : row0 + nrows, 2 : 2 + Wd]
                        wo = (l * 5 + 3) * 384 + m * 128
                        nc.tensor.matmul(
                            g[m][:], wb_sb[:, wo : wo + 128], rhs,
                            start=False, stop=False,
                        )
                        # tap (0,2): K=64 on T1's unshifted half
                        rhs = tv[0:64, row0 : row0 + nrows, 2 : 2 + Wd]
                        wo = (l * 5 + 4) * 384 + m * 128
                        nc.tensor.matmul(
                            g[m][:], wb_sb[0:64, wo : wo + 128], rhs,
                            start=False, stop=True,
                        )

                    # gate nonlinearities: out = func(in + bias).
                    # gate channel ranges inside the 3x128 psum chunks:
                    #   i = g0[0:96], f = g0[96:128] + g1[0:64],
                    #   o = g1[64:128] + g2[0:32], g = g2[32:128]
                    i_s = temps.tile([HID, n], F32, tag="i_s")
                    f_s = temps.tile([HID, n], F32, tag="f_s")
                    o_s = temps.tile([HID, n], F32, tag="o_s")
                    g_t = temps.tile([HID, n], F32, tag="g_t")
                    c_t = temps.tile([HID, n], F32, tag="c_t")
                    ig = temps.tile([HID, n], F32, tag="ig")

                    def act(dst, src_g, gm, p_in0, p_out0, cnt, func):
                        nc.scalar.activation(
                            out=dst[p_out0 : p_out0 + cnt, :],
                            in_=src_g[p_in0 : p_in0 + cnt, :],
                            func=func,
                            bias=b_sb[p_in0 : p_in0 + cnt, bcol[gm] : bcol[gm] + 1],
                        )

                    act(i_s, g[0], 0, 0, 0, 96, ACTF.Sigmoid)
                    act(f_s, g[0], 0, 96, 0, 32, ACTF.Sigmoid)
                    act(f_s, g[1], 1, 0, 32, 32, ACTF.Sigmoid)
                    act(f_s, g[1], 1, 32, 64, 32, ACTF.Sigmoid)
                    act(o_s, g[1], 1, 64, 0, 64, ACTF.Sigmoid)
                    act(o_s, g[2], 2, 0, 64, 32, ACTF.Sigmoid)
                    act(g_t, g[2], 2, 32, 0, 32, ACTF.Tanh)
                    act(g_t, g[2], 2, 64, 32, 32, ACTF.Tanh)
                    act(g_t, g[2], 2, 96, 64, 32, ACTF.Tanh)

                    c_sl = c_sb[l][:, row0 * Wd : row0 * Wd + n]
                    nc.vector.tensor_mul(ig[:], i_s[:], g_t[:])
                    nc.vector.tensor_mul(i_s[:], f_s[:], c_sl)  # i_s := f*c_old
                    nc.vector.tensor_add(c_sl, i_s[:], ig[:])   # c_new
                    nc.scalar.activation(out=c_t[:], in_=c_sl, func=ACTF.Tanh)

                    # h = o * tanh(c), scattered into next-step matmul layouts:
                    #   ch 0:32  -> xz[l][next] partitions 96:128
                    #   ch 32:96 -> tail[l][cur] partitions 0:64 and 64:128
                    #   (l=0 only) full h -> xz[1][cur] partitions 0:96
                    def hmul(dst, p_dst0, p_src0, cnt, c0=1, r_shift=0):
                        r0 = row0 + r_shift
                        drop = -r0 - 1 if r0 < -1 else 0
                        nc.vector.tensor_mul(
                            interior(
                                dst[:], p_dst0, p_dst0 + cnt,
                                nrows=nrows - drop, row0=r0 + drop, c0=c0,
                            ),
                            o_s[p_src0 : p_src0 + cnt, drop * Wd : n],
                            c_t[p_src0 : p_src0 + cnt, drop * Wd : n],
                        )

                    if l == 0:
                        hmul(xz[1][cur], 0, 0, 96)
                    if t < T - 1:
                        hmul(xz[l][prev], 96, 0, 32)
                        hmul(tail[l][cur], 0, 32, 32)
                        hmul(tail[l][cur], 32, 64, 32)
                        hmul(tail[l][cur], 64, 32, 32, c0=0)
                        hmul(tail[l][cur], 96, 64, 32, c0=0)
                        hmul(tail2[l], 0, 32, 32, r_shift=-1)
                        hmul(tail2[l], 32, 64, 32, r_shift=-1)
                        hmul(tail2[l], 64, 32, 32, r_shift=-2)
                        hmul(tail2[l], 96, 64, 32, r_shift=-2)
                    else:
                        # last step: no next-step recurrence consumers; emit h
                        # once to a contiguous staging tile so the final
                        # output store is descriptor-cheap and the tail short
                        h_st = temps.tile([HID, n], F32R, tag="h_st")
                        nc.vector.tensor_mul(h_st[:], o_s[:], c_t[:])
                        nc.sync.dma_start(
                            out=outh_d[l, t, :, row0 * Wd : row0 * Wd + n],
                            in_=h_st[:],
                        )

                # all_hidden[l, t]: ch 0:32 from xz[l][next] p96:128,
                # ch 32:96 from tail[l][cur] p0:64 (last step staged per slab)
                if t < T - 1:
                    nc.sync.dma_start(
                        out=outh_d[l, t, 0:32], in_=interior(xz[l][prev][:], 96, 128)
                    )
                    nc.sync.dma_start(
                        out=outh_d[l, t, 32:96], in_=interior(tail[l][cur][:], 0, 64)
                    )

        for l in range(L):
            nc.sync.dma_start(out=outc_d[l], in_=c_sb[l][:])

    split_multi_waits(nc)
    return nc


_NC = None


def _get_nc():
    global _NC
    if _NC is None:
        _NC = _build()
    return _NC


def make_inmaps(inp, h0, c0, W, b):
    inp = np.ascontiguousarray(inp, dtype=np.float32)  # [8, 12, 96, 48, 48]
    h0 = np.ascontiguousarray(h0, dtype=np.float32)    # [2, 8, 96, 48, 48]
    c0 = np.ascontiguousarray(c0, dtype=np.float32)
    W = np.ascontiguousarray(W, dtype=np.float32)      # [2, 384, 192, 3, 3]
    b = np.ascontiguousarray(b, dtype=np.float32)      # [2, 384]

    # weights -> lhsT layout; wt6: [cin, l, dy, dx, cout]
    wt6 = W.transpose(2, 0, 3, 4, 1)
    w_a = np.ascontiguousarray(wt6[0:128].reshape(128, WAFREE))
    # tail weight groups, free layout (l*5+g)*384+co:
    #   g=0..2: dy-pair -> lower (dy,0), upper (dy,1)
    #   g=3:    T2 pair -> lower (1,2),  upper (2,2)
    #   g=4:    single  -> lower (0,2)
    w_b = np.zeros((128, WBFREE), dtype=np.float32)
    wb5 = w_b.reshape(128, L, 5, 384)
    wb5[0:64, :, 0:3] = wt6[128:192, :, :, 0, :]
    wb5[64:128, :, 0:3] = wt6[128:192, :, :, 1, :]
    wb5[0:64, :, 3] = wt6[128:192, :, 1, 2, :]
    wb5[64:128, :, 3] = wt6[128:192, :, 2, 2, :]
    wb5[0:64, :, 4] = wt6[128:192, :, 0, 2, :]
    # bias -> [partition, (l, cout_chunk)]
    b_r = np.ascontiguousarray(
        b.reshape(L, 3, 128).transpose(2, 0, 1).reshape(128, L * 3)
    )
    zeros = np.zeros((128, PHW), dtype=np.float32)

    def padded(a):  # [ch, 48, 48] -> [ch, 50, 50] zero-padded
        return np.pad(a, ((0, 0), (1, 1), (1, 1)))

    def initial_tiles(bb):
        h0b = h0[:, bb]  # [L, 96, 48, 48]
        xzi = np.zeros((L, 128, H + 2, PW), dtype=np.float32)
        t1i = np.zeros((L, 128, H + 2, PW), dtype=np.float32)
        t2i = np.zeros((L, 128, H + 2, PW), dtype=np.float32)
        xzi[0, 0:96] = padded(inp[bb, 0])
        for l in range(L):
            xzi[l, 96:128] = padded(h0b[l, 0:32])
            tl = padded(h0b[l, 32:96])
            t1i[l, 0:64] = tl
            t1i[l, 64:128, :, 0 : PW - 1] = tl[:, :, 1:PW]      # col-shift 1
            t2i[l, 0:64, 0 : H + 1, :] = tl[:, 1 : H + 2, :]    # row-shift 1
            t2i[l, 64:128, 0:H, :] = tl[:, 2 : H + 2, :]        # row-shift 2
        return (
            xzi.reshape(L, 128, PHW),
            t1i.reshape(L, 128, PHW),
            t2i.reshape(L, 128, PHW),
        )

    inits = [initial_tiles(bb) for bb in range(NCORES)]

    return [
        {
            "inp": inp[bb].reshape(T, C, HW),
            "xzi": inits[bb][0],
            "t1i": inits[bb][1],
            "t2i": inits[bb][2],
            "c0": c0[:, bb].reshape(L, HID, HW),
            "Wa": w_a,
            "Wb": w_b,
            "br": b_r,
            "zeros": zeros,
        }
        for bb in range(NCORES)
    ]


def kernel(inp, h0, c0, W, b):
    in_maps = make_inmaps(inp, h0, c0, W, b)
    res = run_bass_kernel_spmd(_get_nc(), in_maps, list(range(NCORES)))

    all_hidden = np.stack(
        [res.results[bb]["out_h"] for bb in range(NCORES)], axis=1
    ).reshape(L, NCORES, T, HID, H, Wd)
    c_f = np.stack(
        [res.results[bb]["out_c"] for bb in range(NCORES)], axis=1
    ).reshape(L, NCORES, HID, H, Wd)
    h_f = np.ascontiguousarray(all_hidden[:, :, -1])
    return all_hidden, h_f, c_f
